# revision 29
# baseline (speedup 1.0000x reference)
"""MiMoV2 attention (GQA + partial RoPE + attention sinks + causal) on 8 TRN2
NeuronCores.

Sharding: tensor-parallel over heads. Core c owns KV head c and query heads
[4c, 4c+4). Wq/Wk/Wv split along output dim, Wo along input dim; each core
computes a partial output [S, H] which the host sums (the Wo contraction over
heads distributes over cores).

Per-core dataflow (everything head-transposed so no on-chip transposes needed):
  hsT [H, S] streamed by 128-row h-tiles; per s-chunk of 512:
    QT[d, s] (4 heads), KT[d, s] accumulate in PSUM over 32 h-tiles
    V[s, d] natural layout via hsT-as-stationary matmuls
  partial RoPE applied in [d, s] layout; rotate_half is a 32-partition swap
  (cross-partition DVE copies) with the sign folded into sinT.
  scoresT[s_k, s_q] = KT_tile^T @ QT-chunk; exp on ACT (no max subtraction —
  |scores| <= ~12); causal via binary mask multiply on diagonal tiles only.
  attn_outT[d, s_q] accumulates V_tile^T... (lhsT=V tile) @ probsT; softmax
  denominator via all-ones stationary matmul accumulated alongside (sink term
  exp(sink) added per-partition); division by DVE reciprocal + multiply.
  out_partial[s, o] = attnT-as-stationary @ Wo-chunk, accumulated over the 4
  local heads; written out as bf16; host sums the 8 partials in fp32.
"""

import numpy as np
import ml_dtypes
from contextlib import ExitStack

import concourse.bass as bass
import concourse.mybir as mybir
import concourse.tile as tile
from concourse.bass_utils import run_bass_kernel_spmd

bf16 = ml_dtypes.bfloat16
BF = mybir.dt.bfloat16
F32 = mybir.dt.float32

N_CORES = 8
S = 2048
H = 4096
HD = 128
ROPE = 64
NHL = 4                    # local query heads per core
CH = 512                   # s-chunk width
NCHUNK = S // CH           # 4
HT = H // 128              # 32 h-tiles
NKT = S // 128             # 16 k-tiles

# this walrus build allows at most one sync wait per instruction
_MAX_WAITS = 1


def _split_excess_waits(nc):
    cnt = 0
    for f in nc.m.functions:
        for bb in f.blocks:
            out, changed = [], False
            for inst in bb.instructions:
                si = inst.sync_info
                if si is not None and len(si.on_wait) > _MAX_WAITS:
                    waits = list(si.on_wait)
                    excess, keep = waits[:-_MAX_WAITS], waits[-_MAX_WAITS:]
                    for i in range(0, len(excess), _MAX_WAITS):
                        cnt += 1
                        out.append(mybir.InstNoOp(
                            name=f"waitnop-{cnt}", engine=inst.engine,
                            sync_info=mybir.SyncInfo(
                                on_wait=excess[i:i + _MAX_WAITS], on_update=[])))
                    si.on_wait = keep
                    changed = True
                out.append(inst)
            if changed:
                bb.instructions = out
    return cnt


def _rope_copy(nc, pool, psum_t, dest, cos_sb, sin_sb, sl):
    """psum_t [128,512] fp32 -> dest [128,512] bf16 slice, applying partial
    RoPE to rows 0:64 (rotate_half = +-32-partition swap, sign pre-folded
    into sin_sb)."""
    # pass-through rows 64:128 on ACT (keeps DVE free)
    nc.scalar.copy(dest[64:128, :], psum_t[64:128, :])
    # swapped copy of the rotary rows (cross-partition reads from PSUM)
    sw = pool.tile([64, CH], BF, tag="rope_sw")
    nc.vector.tensor_copy(sw[0:32, :], psum_t[32:64, :])
    nc.vector.tensor_copy(sw[32:64, :], psum_t[0:32, :])
    # t1 = q_r * cos   (one fused op: (psum mult 1.0) mult cos)
    t1 = pool.tile([64, CH], BF, tag="rope_t1")
    nc.vector.scalar_tensor_tensor(
        t1[:, :], psum_t[0:64, :], 1.0, cos_sb[:, sl],
        op0=mybir.AluOpType.mult, op1=mybir.AluOpType.mult)
    t2 = pool.tile([64, CH], BF, tag="rope_t2")
    nc.vector.tensor_mul(t2[:, :], sw[:, :], sin_sb[:, sl])
    nc.vector.tensor_add(dest[0:64, :], t1[:, :], t2[:, :])


def build_bass(repeat=1):
    """repeat>1 duplicates the whole compute body (for timing: the wall-clock
    delta between repeat=2 and repeat=1 NEFFs is one kernel iteration,
    independent of the large fixed PJRT/axon dispatch overhead)."""
    nc = bass.Bass("TRN2", target_bir_lowering=False, debug=False)

    hsT = nc.dram_tensor("hsT", [H, S], BF, kind="ExternalInput")
    wq = nc.dram_tensor("wq", [H, NHL * HD], BF, kind="ExternalInput")
    wk = nc.dram_tensor("wk", [H, HD], BF, kind="ExternalInput")
    wv = nc.dram_tensor("wv", [H, HD], BF, kind="ExternalInput")
    wo = nc.dram_tensor("wo", [NHL * HD, H], BF, kind="ExternalInput")
    cosT = nc.dram_tensor("cosT", [ROPE, S], BF, kind="ExternalInput")
    sinTs = nc.dram_tensor("sinTs", [ROPE, S], BF, kind="ExternalInput")
    esink = nc.dram_tensor("esink", [NHL, 128], F32, kind="ExternalInput")
    maskb = nc.dram_tensor("maskb", [128, 1024], BF, kind="ExternalInput")
    outp = nc.dram_tensor("outp", [S, H], BF, kind="ExternalOutput")

    with tile.TileContext(nc) as tc, ExitStack() as ctx:
        const = ctx.enter_context(tc.tile_pool(name="const", bufs=1))
        hs_pool = ctx.enter_context(tc.tile_pool(name="hs", bufs=9))
        rope_pool = ctx.enter_context(tc.tile_pool(name="rope", bufs=2))
        probs_pool = ctx.enter_context(tc.tile_pool(name="probs", bufs=6))
        den_pool = ctx.enter_context(tc.tile_pool(name="den", bufs=2))
        out_pool = ctx.enter_context(tc.tile_pool(name="out", bufs=2))

        # ---- constants / weights resident in SBUF ----
        # weights are loaded in h-tile groups so the first projection matmuls
        # only wait on the first slice, not the whole 4MB tensor
        wq_sb = const.tile([128, HT, NHL * HD], BF)
        wk_sb = const.tile([128, HT, HD], BF)
        wv_sb = const.tile([128, HT, HD], BF)
        wq_r = wq.rearrange("(t p) c -> p t c", p=128)
        wk_r = wk.rearrange("(t p) c -> p t c", p=128)
        wv_r = wv.rearrange("(t p) c -> p t c", p=128)
        hsT_r = hsT.rearrange("(t p) s -> p t s", p=128)
        wo_sb = const.tile([128, NHL, H], BF)
        cos_sb = const.tile([ROPE, S], BF)
        nc.gpsimd.dma_start(out=cos_sb, in_=cosT[:, :])
        sin_sb = const.tile([ROPE, S], BF)
        nc.gpsimd.dma_start(out=sin_sb, in_=sinTs[:, :])
        mask_sb = const.tile([128, 1024], BF)
        nc.gpsimd.dma_start(out=mask_sb, in_=maskb[:, :])
        esink_sb = const.tile([128, NHL], F32)
        for h in range(NHL):
            nc.gpsimd.dma_start(out=esink_sb[:, h:h + 1],
                              in_=esink[h].rearrange("(p c) -> p c", c=1))
        ones_sb = const.tile([128, 128], BF)
        nc.vector.memset(ones_sb[:, :], 1.0)

        # persistent activations
        qt_sb = const.tile([128, NHL, S], BF)     # QT per head [d, s]
        kt_sb = const.tile([128, S], BF)          # KT [d, s]
        vt_sb = const.tile([128, S], BF)          # VT [d, s] (pre-transpose)
        v_sb = const.tile([128, NKT, HD], BF)     # V [s(128), kt, d]
        at_sb = const.tile([128, NHL, S], BF)     # attnT per head [d, s]

        for _rep in range(repeat):
            # phases 1+2 share one PSUM scope (8 banks: proj 2 + ps 2 + po 2
            # + pd 2) so projection chunks and attention chunks interleave on
            # PE with no pool-boundary serialization.
            with ExitStack() as p12:
                proj_pool = p12.enter_context(
                    tc.tile_pool(name="proj", bufs=3, space="PSUM"))
                ps_pool = p12.enter_context(
                    tc.tile_pool(name="ps", bufs=2, space="PSUM"))
                po_pool = p12.enter_context(
                    tc.tile_pool(name="po", bufs=2, space="PSUM"))
                pd_pool = p12.enter_context(
                    tc.tile_pool(name="pd", bufs=1, space="PSUM"))

                def emit_p1(ci, load_weights=False):
                    """QKV projections + RoPE for s-chunk ci. The whole hsT
                    chunk (32 tiles) is resident, so each projection output is
                    one contiguous 32-MM accumulation group in a single PSUM
                    bank (groups never interleave within a bank)."""
                    sl = bass.ds(ci * CH, CH)
                    hs4 = []
                    for g4 in range(HT // 4):
                        if load_weights and g4 % 2 == 0:
                            # interleave weight-slice loads with the hst
                            # stream so the first matmuls start early
                            g = g4 * 4
                            nc.sync.dma_start(out=wq_sb[:, g:g + 8, :],
                                              in_=wq_r[:, g:g + 8, :])
                            nc.sync.dma_start(out=wk_sb[:, g:g + 8, :],
                                              in_=wk_r[:, g:g + 8, :])
                            nc.sync.dma_start(out=wv_sb[:, g:g + 8, :],
                                              in_=wv_r[:, g:g + 8, :])
                        h4 = hs_pool.tile([128, 4, CH], BF, tag="hst",
                                          name=f"hst_{_rep}_{ci}_{g4}")
                        nc.sync.dma_start(
                            out=h4, in_=hsT_r[:, g4 * 4:(g4 + 1) * 4, sl])
                        hs4.append(h4)
                    hts = [hs4[t // 4][:, t % 4, :] for t in range(HT)]

                    def copy_out(pp, rope):
                        if rope is not None:
                            _rope_copy(nc, rope_pool, pp, rope, cos_sb, sin_sb, sl)
                        else:
                            nc.vector.tensor_copy(vt_sb[:, sl], pp[:, :])
                            for st in range(4):
                                kj = ci * 4 + st
                                nc.sync.dma_start_transpose(
                                    out=v_sb[:, kj, :],
                                    in_=vt_sb[:, kj * 128:(kj + 1) * 128])

                    groups = [
                        (lambda t, h=h: wq_sb[:, t, h * HD:(h + 1) * HD],
                         qt_sb[:, h, sl], f"q{h}") for h in range(NHL)
                    ] + [
                        (lambda t: wk_sb[:, t, :], kt_sb[:, sl], "k"),
                        (lambda t: wv_sb[:, t, :], None, "v"),
                    ]

                    if load_weights:
                        # chunk 0 is paced by the input DMA stream: interleave
                        # 3 groups across arriving hst tiles (3 PSUM banks) so
                        # PE keeps up with the DMA rate instead of idling
                        for trip in (groups[0:3], groups[3:6]):
                            pps = [proj_pool.tile(
                                [128, CH], F32, tag="pp",
                                name=f"pp_{_rep}_{ci}_{d}") for _, _, d in trip]
                            for t in range(HT):
                                for gi in range(3):
                                    nc.tensor.matmul(
                                        pps[gi][:, :], trip[gi][0](t),
                                        hts[t][:, :],
                                        start=(t == 0), stop=(t == HT - 1))
                            for gi in range(3):
                                copy_out(pps[gi], trip[gi][1])
                    else:
                        for lhs_of_t, rope, dest in groups:
                            pp = proj_pool.tile([128, CH], F32,
                                                name=f"pp_{_rep}_{ci}_{dest}",
                                                tag="pp")
                            for t in range(HT):
                                nc.tensor.matmul(pp[:, :], lhs_of_t(t),
                                                 hts[t][:, :],
                                                 start=(t == 0),
                                                 stop=(t == HT - 1))
                            copy_out(pp, rope)

                def emit_p2(ci):
                    """Attention for query chunk ci, all 4 local heads.
                    Emission is software-pipelined: scores(kj+1) is emitted
                    before attnV(kj) so PE computes the next score tile while
                    ACT does exp of the previous one."""
                    q0 = ci * CH
                    n_kt = 4 * (ci + 1)
                    for h in range(NHL):
                        po = po_pool.tile([128, CH], F32,
                                          name=f"po_{_rep}_{ci}_{h}", tag="po")
                        pd = pd_pool.tile([128, CH], F32,
                                          name=f"pd_{_rep}_{ci}_{h}", tag="pd")
                        stage = []  # (kj, ps, pr, off)

                        def emit_scores(kj):
                            off = kj * 128 - q0
                            ps = ps_pool.tile([128, CH], F32,
                                              name=f"ps_{_rep}_{ci}_{h}_{kj}",
                                              tag="ps")
                            kt_t = kt_sb[:, kj * 128:(kj + 1) * 128]
                            if off > 0:
                                # columns < off are fully masked: skip them
                                nc.tensor.matmul(ps[:, off:],
                                                 kt_t, qt_sb[:, h, q0 + off:q0 + CH],
                                                 start=True, stop=True)
                            else:
                                nc.tensor.matmul(ps[:, :], kt_t,
                                                 qt_sb[:, h, q0:q0 + CH],
                                                 start=True, stop=True)
                            pr = probs_pool.tile([128, CH], BF,
                                                 name=f"pr_{_rep}_{ci}_{h}_{kj}",
                                                 tag="pr")
                            if off > 0:
                                nc.gpsimd.memset(pr[:, 0:off], 0.0)
                                nc.scalar.activation(
                                    pr[:, off:], ps[:, off:],
                                    mybir.ActivationFunctionType.Exp)
                            else:
                                nc.scalar.activation(
                                    pr[:, :], ps[:, :],
                                    mybir.ActivationFunctionType.Exp)
                            if off >= 0:
                                # triangular 128-col band at q_local in
                                # [off, off+128): maskb[:, 512:640] is the
                                # aligned triangle for every diagonal tile
                                nc.vector.tensor_mul(
                                    pr[:, off:off + 128], pr[:, off:off + 128],
                                    mask_sb[:, 512:640])
                            stage.append((kj, ps, pr, off))

                        def emit_av():
                            kj, ps, pr, off = stage.pop(0)
                            fl = dict(start=(kj == 0), stop=(kj == n_kt - 1))
                            if off > 0:
                                nc.tensor.matmul(po[:, off:], v_sb[:, kj, :],
                                                 pr[:, off:], **fl)
                                nc.tensor.matmul(pd[:, off:], ones_sb[:, :],
                                                 pr[:, off:], **fl)
                            else:
                                nc.tensor.matmul(po[:, :], v_sb[:, kj, :],
                                                 pr[:, :], **fl)
                                nc.tensor.matmul(pd[:, :], ones_sb[:, :],
                                                 pr[:, :], **fl)

                        emit_scores(0)
                        for kj in range(1, n_kt):
                            emit_scores(kj)
                            emit_av()
                        emit_av()

                        den = den_pool.tile([128, CH], F32, tag="den",
                                            name=f"den_{_rep}_{ci}_{h}")
                        nc.vector.tensor_scalar_add(den[:, :], pd[:, :],
                                                    esink_sb[:, h:h + 1])
                        rec = den_pool.tile([128, CH], F32, tag="rec",
                                            name=f"rec_{_rep}_{ci}_{h}")
                        nc.vector.reciprocal(rec[:, :], den[:, :])
                        nc.vector.tensor_mul(at_sb[:, h, q0:q0 + CH],
                                             po[:, :], rec[:, :])

                def emit_p3(ci, pool, bufs_tag):
                    """Output projection for the 4 s-tiles of chunk ci."""
                    for st in range(ci * 4, (ci + 1) * 4):
                        ob = out_pool.tile([128, H], BF, tag="ob",
                                           name=f"ob_{_rep}_{st}")
                        for oc in range(H // CH):
                            pw = pool.tile([128, CH], F32, tag=bufs_tag,
                                           name=f"pw_{_rep}_{st}_{oc}")
                            for h in range(NHL):
                                nc.tensor.matmul(
                                    pw[:, :],
                                    at_sb[:, h, st * 128:(st + 1) * 128],
                                    wo_sb[:, h, oc * CH:(oc + 1) * CH],
                                    start=(h == 0), stop=(h == NHL - 1))
                            # alternate copy engine to split PSUM->SBUF load
                            if (st * (H // CH) + oc) % 2 == 0:
                                nc.vector.tensor_copy(
                                    ob[:, oc * CH:(oc + 1) * CH], pw[:, :])
                            else:
                                nc.scalar.copy(
                                    ob[:, oc * CH:(oc + 1) * CH], pw[:, :])
                        nc.sync.dma_start(
                            out=outp[st * 128:(st + 1) * 128, :], in_=ob[:, :])

                # interleave: projections stay ~1 chunk ahead of attention so
                # PE never waits on the rope copy chain; after the last
                # projection chunk, output-projection chunks reuse the proj
                # pool's PSUM slots (tag "pp") to overlap with attention.
                wo_r = wo.rearrange("(t p) c -> p t c", p=128)
                emit_p1(0, load_weights=(_rep == 0))
                emit_p1(1)
                emit_p2(0)
                emit_p1(2)
                if _rep == 0:
                    nc.sync.dma_start(out=wo_sb[:, 0:2, :], in_=wo_r[:, 0:2, :])
                    nc.sync.dma_start(out=wo_sb[:, 2:4, :], in_=wo_r[:, 2:4, :])
                emit_p2(1)
                emit_p1(3)
                emit_p3(0, proj_pool, "pp")
                emit_p2(2)
                emit_p3(1, proj_pool, "pp")
                emit_p2(3)
                emit_p3(2, proj_pool, "pp")
                emit_p3(3, proj_pool, "pp")

    _split_excess_waits(nc)
    return nc


_NC_CACHE = None


def _get_nc():
    global _NC_CACHE
    if _NC_CACHE is None:
        _NC_CACHE = build_bass()
    return _NC_CACHE


def make_in_maps(hidden_states, cos, sin, Wq, Wk, Wv, Wo, sinks):
    scaling = HD ** -0.5
    hs = np.asarray(hidden_states, dtype=np.float32).reshape(S, H)
    hsT = np.ascontiguousarray(hs.T).astype(bf16)
    cosT = np.ascontiguousarray(np.asarray(cos, np.float32).reshape(S, ROPE).T)
    sinT = np.ascontiguousarray(np.asarray(sin, np.float32).reshape(S, ROPE).T)
    sinTs = sinT.copy()
    sinTs[:ROPE // 2] *= -1.0
    cosT = cosT.astype(bf16)
    sinTs = sinTs.astype(bf16)
    Wq = np.asarray(Wq, np.float32)
    Wk = np.asarray(Wk, np.float32)
    Wv = np.asarray(Wv, np.float32)
    Wo = np.asarray(Wo, np.float32)
    sinks = np.asarray(sinks, np.float32)
    maskb = ((np.arange(1024)[None, :] - 512) >= np.arange(128)[:, None])
    maskb = maskb.astype(np.float32).astype(bf16)

    in_maps = []
    for c in range(N_CORES):
        qcols = slice(NHL * HD * c, NHL * HD * (c + 1))
        esink_c = np.exp(sinks[NHL * c:NHL * (c + 1)]).astype(np.float32)
        in_maps.append({
            "hsT": hsT,
            "wq": np.ascontiguousarray(Wq[:, qcols] * scaling).astype(bf16),
            "wk": np.ascontiguousarray(Wk[:, HD * c:HD * (c + 1)]).astype(bf16),
            "wv": np.ascontiguousarray(Wv[:, HD * c:HD * (c + 1)]).astype(bf16),
            "wo": np.ascontiguousarray(Wo[qcols, :]).astype(bf16),
            "cosT": cosT,
            "sinTs": sinTs,
            "esink": np.repeat(esink_c[:, None], 128, axis=1).copy(),
            "maskb": maskb,
        })
    return in_maps


def kernel(hidden_states, cos, sin, attention_mask, Wq, Wk, Wv, Wo, sinks):
    # attention_mask is the standard causal mask; causality is built into the
    # kernel (binary masks on the diagonal score tiles), so it is unused.
    in_maps = make_in_maps(hidden_states, cos, sin, Wq, Wk, Wv, Wo, sinks)
    nc = _get_nc()
    res = run_bass_kernel_spmd(nc, in_maps, core_ids=list(range(N_CORES)))
    acc = np.zeros((S, H), dtype=np.float32)
    for r in res.results:
        acc += r["outp"].astype(np.float32)
    return acc.reshape(1, S, H)
